# revision 9
# baseline (speedup 1.0000x reference)
"""GyroLoss Trainium2 kernel.

Self-contained: takes FULL inputs xs, hat_xs [64, 32768, 3] f32, returns the
scalar f32 loss, matching the reference GyroLoss (target='rotation matrix').

Strategy (data-parallel over batch, 8 rows/core on 8 cores):
  - Rotations are tracked as UNNORMALIZED quaternions in SoA "plane" layout.
    Unnormalized (projective) quats avoid all divisions until the tiny log
    stage: q = (n*cos(h), sin(h)*v) for phi = s*v, h = (s/2)*n, n = |v|.
  - The 4/5-level pair-reduction tree works on halves of a bit-reversed
    element layout (host-side permutation), so every operand of every tree
    level is a contiguous/affine slice.
  - Level buffers use a [w|x|y|z|x|y] 6-block layout so the quaternion
    product needs only 10 vector instructions per level.
  - log: c=(w^2-n2)/(w^2+n2) clipped, arccos by Hastings poly * sqrt(1-|c|),
    |rs_c| = theta*|v_c|/|v|; Huber = 0.5*min(u,1)^2 + relu(u-1); the
    "drop first N0 per row" is a 0/1 mask folded into one multiply.
  - Per-core output: [128, 2] per-partition partial sums; host combines.
"""

import sys

import numpy as np

for _p in ("/opt/trn_rl_repo",):
    if _p not in sys.path:
        sys.path.append(_p)

import concourse.bass as bass
import concourse.tile as tile
from concourse import mybir
from concourse.bass_utils import run_bass_kernel_spmd

AF = mybir.ActivationFunctionType
OP = mybir.AluOpType
F32 = mybir.dt.float32
BF16 = mybir.dt.bfloat16

N_CORES = 8
ROWS_PER_CORE = 8
T = 2048            # hat times per partition
T4 = 128            # level-4 elements per partition
N0 = 5
HUBER = 0.005
W_CONST = 1e6
CNT4 = 64 * 2043 * 3
CNT5 = 64 * 1019 * 3
PI = float(np.pi)
EPS_CLIP = 1e-7

# knobs
TREE_DT = F32       # dtype of quaternion planes / tree math
N_CHUNKS = 4        # DMA/exp chunks over the 2048 columns

# Hastings/Abramowitz-Stegun 4.4.45 arccos coefficients (c0..c7)
ACOS_C = [1.5707963050, -0.2145988016, 0.0889789874, -0.0501743046,
          0.0308918810, -0.0170881256, 0.0066700901, -0.0012624911]


# ---------------------------------------------------------------- host layout
def _bitrev5(u):
    r = 0
    for i in range(5):
        r |= ((u >> i) & 1) << (4 - i)
    return r


def _perm_t():     # position of time t within a partition's 2048 columns
    t = np.arange(T)
    g = t >> 5
    u = t & 31
    urev = np.array([_bitrev5(int(x)) for x in u])
    return urev * 64 + g


def _perm_t4():    # position of level-4 element t4 within 128 columns
    t4 = np.arange(T4)
    return (t4 & 1) * 64 + (t4 >> 1)


F_OF_T = _perm_t()
F4_OF_T4 = _perm_t4()


def _host_masks():
    mask = np.ones((128, 192), np.float32)
    pp = np.arange(128) % 16 == 0
    mask4 = np.ones((128, 128), np.float32)
    mask4[np.ix_(pp, F4_OF_T4[:N0])] = 0.0
    mask5 = np.ones((128, 64), np.float32)
    mask5[pp, :N0] = 0.0
    mask[:, :128] = mask4
    mask[:, 128:] = mask5
    return mask


# ---------------------------------------------------------------- bass builder
def _emit_exp(nc, pool, ph, qd, col0, width, half_scale, tag):
    """phi planes ph [128,3,W] f32 -> unnormalized quat into 6-block qd tile
    at columns [col0, col0+width). q = (n*cos(h), sin(h)*v), h = half_scale*n,
    cos(h) = 1 - 2*sin(h/2)^2 (keeps Sin args small)."""
    v = nc.vector
    a = nc.scalar
    nb = 2
    sq = pool.tile([128, 3, width], F32, tag=f"{tag}_sq", name=f"{tag}_sq",
                   bufs=nb)
    nn = pool.tile([128, width], F32, tag=f"{tag}_n", name=f"{tag}_n",
                   bufs=nb)
    sh = pool.tile([128, width], F32, tag=f"{tag}_sh", name=f"{tag}_sh",
                   bufs=nb)
    s2 = pool.tile([128, width], F32, tag=f"{tag}_s2", name=f"{tag}_s2",
                   bufs=nb)

    for c in range(3):
        a.activation(sq[:, c, :], ph[:, c, :], AF.Square)
    v.tensor_tensor(nn[:], sq[:, 0, :], sq[:, 1, :], OP.add)
    v.tensor_tensor(nn[:], nn[:], sq[:, 2, :], OP.add)
    a.activation(nn[:], nn[:], AF.Sqrt)
    a.activation(sh[:], nn[:], AF.Sin, scale=half_scale)
    a.activation(s2[:], nn[:], AF.Sin, scale=half_scale * 0.5)
    a.activation(s2[:], s2[:], AF.Square)
    v.tensor_scalar(s2[:], s2[:], -2.0, 1.0, OP.mult, OP.add)

    cols = slice(col0, col0 + width)
    v.tensor_tensor(qd[:, 0, cols], nn[:], s2[:], OP.mult)
    sh3 = sh[:].unsqueeze(1).broadcast_to([128, 3, width])
    v.tensor_tensor(qd[:, 1:4, cols], sh3, ph[:, :, :], OP.mult)
    a.activation(qd[:, 4:6, cols], qd[:, 1:3, cols], AF.Copy)


def _emit_qprod(nc, pool, A, B, out, L, tag, conj_a=False, terminal=False):
    """out = (conj(A) if conj_a else A) (x) B, quaternion product on planes.
    A, B: [128, 6, L] APs in [w|x|y|z|x|y] block layout.
    out: [128, 6, L] tile (or [128, 4, L] if terminal: no appends emitted).
    """
    v = nc.vector
    a = nc.scalar
    aw3 = A[:, 0, :].unsqueeze(1).broadcast_to([128, 3, L])
    bw3 = B[:, 0, :].unsqueeze(1).broadcast_to([128, 3, L])

    t1 = pool.tile([128, 3, L], TREE_DT, tag="qp_t1", name=f"qp_t1_{tag}")
    cr = pool.tile([128, 3, L], TREE_DT, tag="qp_cr", name=f"qp_cr_{tag}")
    mm = pool.tile([128, 4, L], TREE_DT, tag="qp_mm", name=f"qp_mm_{tag}")
    s4 = pool.tile([128, L], TREE_DT, tag="qp_s4", name=f"qp_s4_{tag}")

    # cvec = aw*bv +/- bw*av +/- (rot1(a)*rot2(b) - rot2(a)*rot1(b))
    sgn1 = OP.subtract if conj_a else OP.add
    sgn2 = OP.add if conj_a else OP.subtract
    v.tensor_tensor(t1[:], aw3, B[:, 1:4, :], OP.mult)
    v.tensor_tensor(cr[:], bw3, A[:, 1:4, :], OP.mult)
    v.tensor_tensor(t1[:], t1[:], cr[:], sgn1)
    v.tensor_tensor(cr[:], A[:, 2:5, :], B[:, 3:6, :], OP.mult)
    v.tensor_tensor(t1[:], t1[:], cr[:], sgn1)
    v.tensor_tensor(cr[:], A[:, 3:6, :], B[:, 2:5, :], OP.mult)
    v.tensor_tensor(out[:, 1:4, :], t1[:], cr[:], sgn2)
    # cw
    v.tensor_tensor(mm[:], A[:, 0:4, :], B[:, 0:4, :], OP.mult)
    v.tensor_reduce(s4[:], mm[:].transpose([0, 2, 1]), mybir.AxisListType.X,
                    OP.add)
    if conj_a:
        v.tensor_copy(out=out[:, 0, :], in_=s4[:])
    else:
        v.scalar_tensor_tensor(out[:, 0, :], mm[:, 0, :], 2.0, s4[:],
                               OP.mult, OP.subtract)
    if not terminal:
        a.activation(out[:, 4:6, :], out[:, 1:3, :], AF.Copy)


def _emit_log_huber(nc, pool, r, maskap, L, accs, tag):
    """r [128,4,L] quat planes (f32) -> per-partition huber sums into
    accs[c] [128,1] for c in 0..2."""
    v = nc.vector
    a = nc.scalar
    w2 = pool.tile([128, L], F32, tag=f"{tag}_w2")
    sq = pool.tile([128, 3, L], F32, tag=f"{tag}_sq")
    n2a = pool.tile([128, L], F32, tag=f"{tag}_n2a")
    n2 = pool.tile([128, L], F32, tag=f"{tag}_n2")
    den = pool.tile([128, L], F32, tag=f"{tag}_den")
    num = pool.tile([128, L], F32, tag=f"{tag}_num")
    rec = pool.tile([128, L], F32, tag=f"{tag}_rec")
    cc = pool.tile([128, L], F32, tag=f"{tag}_cc")
    acl = pool.tile([128, L], F32, tag=f"{tag}_acl")
    u1 = pool.tile([128, L], F32, tag=f"{tag}_u1")
    sq1 = pool.tile([128, L], F32, tag=f"{tag}_sq1")
    base = pool.tile([128, L], F32, tag=f"{tag}_base")
    sg = pool.tile([128, L], F32, tag=f"{tag}_sg")
    th = pool.tile([128, L], F32, tag=f"{tag}_th")
    n2c = pool.tile([128, L], F32, tag=f"{tag}_n2c")
    rin = pool.tile([128, L], F32, tag=f"{tag}_rin")
    g2 = pool.tile([128, L], F32, tag=f"{tag}_g2")

    a.activation(w2[:], r[:, 0, :], AF.Square)
    for c in range(3):
        a.activation(sq[:, c, :], r[:, 1 + c, :], AF.Square)
    v.tensor_tensor(n2a[:], sq[:, 0, :], sq[:, 1, :], OP.add)
    v.tensor_tensor(n2[:], n2a[:], sq[:, 2, :], OP.add)
    v.tensor_tensor(den[:], w2[:], n2[:], OP.add)
    v.tensor_tensor(num[:], w2[:], n2[:], OP.subtract)
    v.reciprocal(rec[:], den[:])
    v.tensor_tensor(cc[:], num[:], rec[:], OP.mult)
    v.tensor_scalar(cc[:], cc[:], 1.0 - EPS_CLIP, -1.0 + EPS_CLIP,
                    OP.min, OP.max)
    a.activation(acl[:], cc[:], AF.Abs)
    # Hastings: arccos(|c|) = sqrt(1-|c|) * P(|c|)
    v.tensor_scalar(u1[:], acl[:], ACOS_C[7], None, OP.mult)
    for k in range(6, 0, -1):
        v.scalar_tensor_tensor(u1[:], u1[:], ACOS_C[k], acl[:],
                               OP.add, OP.mult)
    a.activation(sq1[:], acl[:], AF.Sqrt, bias=1.0, scale=-1.0)
    v.scalar_tensor_tensor(base[:], u1[:], ACOS_C[0], sq1[:], OP.add, OP.mult)
    a.activation(sg[:], cc[:], AF.Sign)
    # theta = sign(c)*(base - pi/2) + pi/2
    v.tensor_scalar(base[:], base[:], -PI / 2, None, OP.add)
    v.tensor_tensor(th[:], sg[:], base[:], OP.mult)
    v.tensor_scalar(th[:], th[:], PI / 2, None, OP.add)
    v.tensor_scalar(n2c[:], n2[:], 1e-30, None, OP.max)
    v.reciprocal(n2c[:], n2c[:])
    a.activation(rin[:], n2c[:], AF.Sqrt)
    v.tensor_tensor(th[:], th[:], rin[:], OP.mult)
    v.scalar_tensor_tensor(g2[:], th[:], 1.0 / HUBER, maskap, OP.mult, OP.mult)

    for c in range(3):
        av = pool.tile([128, L], F32, tag=f"{tag}_av")
        uu = pool.tile([128, L], F32, tag=f"{tag}_uu")
        mi = pool.tile([128, L], F32, tag=f"{tag}_mi")
        m2 = pool.tile([128, L], F32, tag=f"{tag}_m2")
        ru = pool.tile([128, L], F32, tag=f"{tag}_ru")
        hh = pool.tile([128, L], F32, tag=f"{tag}_hh")
        a.activation(av[:], r[:, 1 + c, :], AF.Abs)
        v.tensor_tensor(uu[:], av[:], g2[:], OP.mult)
        v.tensor_scalar(mi[:], uu[:], 1.0, None, OP.min)
        a.activation(m2[:], mi[:], AF.Square)
        v.tensor_scalar(ru[:], uu[:], -1.0, 0.0, OP.add, OP.max)
        v.scalar_tensor_tensor(hh[:], m2[:], 0.5, ru[:], OP.mult, OP.add,
                               accum_out=accs[c][:])


def _split_multiwaits(nc, max_waits=1):
    """The walrus codegen on this toolchain accepts at most one sync-wait per
    instruction; hoist extra waits onto injected same-engine NoOps."""
    nid = 0
    for f in nc.m.functions:
        for bb in f.blocks:
            newlist = []
            for ins in bb.instructions:
                si = ins.sync_info
                if si is not None and si.on_wait and len(si.on_wait) > max_waits:
                    extra = si.on_wait[:-max_waits]
                    keep = si.on_wait[-max_waits:]
                    for wt in extra:
                        nid += 1
                        nop = mybir.InstNoOp(name=f"WSPLIT-{nid}",
                                             engine=ins.engine)
                        nop.sync_info = mybir.SyncInfo(on_wait=[wt],
                                                       on_update=[])
                        newlist.append(nop)
                    ins.sync_info = mybir.SyncInfo(
                        on_wait=list(keep), on_update=list(si.on_update))
                newlist.append(ins)
            bb.instructions[:] = newlist


def build_nc():
    nc = bass.Bass()
    phi_d = nc.declare_dram_parameter("phi", [128, 3, T], F32, isOutput=False)
    xphi_d = nc.declare_dram_parameter("xphi", [128, 3, T4], F32,
                                       isOutput=False)
    mask_d = nc.declare_dram_parameter("mask", [128, 192], F32, isOutput=False)
    out_d = nc.declare_dram_parameter("out", [128, 2], F32, isOutput=True)

    with tile.TileContext(nc) as tc:
        with tc.tile_pool(name="main", bufs=1) as pool, \
             tc.tile_pool(name="chunks", bufs=2) as cpool:
            # ---- X side
            xp = pool.tile([128, 3, T4], F32, tag="xp")
            nc.sync.dma_start(out=xp[:], in_=xphi_d[:])
            mt = pool.tile([128, 192], F32, tag="mt")
            nc.sync.dma_start(out=mt[:], in_=mask_d[:])
            xq = pool.tile([128, 6, T4], TREE_DT, tag="xq")
            _emit_exp(nc, pool, xp[:], xq[:], 0, T4, 0.5, "xexp")

            # ---- Omega exp (chunked DMA)
            q0 = pool.tile([128, 6, T], TREE_DT, tag="q0")
            cw = T // N_CHUNKS
            for c in range(N_CHUNKS):
                ph = cpool.tile([128, 3, cw], F32, tag="phchunk")
                nc.sync.dma_start(out=ph[:],
                                  in_=phi_d[:, :, c * cw:(c + 1) * cw])
                _emit_exp(nc, pool, ph[:], q0[:], c * cw, cw, 0.005, "oexp")

            # ---- tree
            q1 = pool.tile([128, 6, 1024], TREE_DT, tag="qodd", name="q1")
            q2 = pool.tile([128, 6, 512], TREE_DT, tag="qeven", name="q2")
            q3 = pool.tile([128, 6, 256], TREE_DT, tag="qodd", name="q3")
            q4 = pool.tile([128, 6, 128], TREE_DT, tag="qeven", name="q4")
            q5 = pool.tile([128, 6, 64], TREE_DT, tag="q5")
            x5 = pool.tile([128, 6, 64], TREE_DT, tag="x5")
            _emit_qprod(nc, pool, q0[:, :, 0:1024], q0[:, :, 1024:2048],
                        q1[:], 1024, "l1")
            _emit_qprod(nc, pool, q1[:, :, 0:512], q1[:, :, 512:1024],
                        q2[:], 512, "l2")
            _emit_qprod(nc, pool, q2[:, :, 0:256], q2[:, :, 256:512],
                        q3[:], 256, "l3")
            _emit_qprod(nc, pool, q3[:, :, 0:128], q3[:, :, 128:256],
                        q4[:], 128, "l4")
            _emit_qprod(nc, pool, q4[:, :, 0:64], q4[:, :, 64:128],
                        q5[:], 64, "l5")
            _emit_qprod(nc, pool, xq[:, :, 0:64], xq[:, :, 64:128],
                        x5[:], 64, "x5")

            # ---- bmtm: r = conj(Omega) (x) X
            r4 = pool.tile([128, 4, 128], F32, tag="r4")
            r5 = pool.tile([128, 4, 64], F32, tag="r5")
            _emit_qprod(nc, pool, q4[:], xq[:], r4[:], 128, "b4",
                        conj_a=True, terminal=True)
            _emit_qprod(nc, pool, q5[:], x5[:], r5[:], 64, "b5",
                        conj_a=True, terminal=True)

            # ---- log + huber + accumulate
            accs4 = [pool.tile([128, 1], F32, tag=f"acc4_{c}", name=f"acc4_{c}")
                     for c in range(3)]
            accs5 = [pool.tile([128, 1], F32, tag=f"acc5_{c}", name=f"acc5_{c}")
                     for c in range(3)]
            _emit_log_huber(nc, pool, r4[:], mt[:, 0:128], 128, accs4, "h4")
            _emit_log_huber(nc, pool, r5[:], mt[:, 128:192], 64, accs5, "h5")

            # ---- combine and store
            ot = pool.tile([128, 2], F32, tag="ot")
            tmp = pool.tile([128, 1], F32, tag="sumtmp")
            nc.vector.tensor_tensor(tmp[:], accs4[0][:], accs4[1][:], OP.add)
            nc.vector.tensor_tensor(ot[:, 0:1], tmp[:], accs4[2][:], OP.add)
            nc.vector.tensor_tensor(tmp[:], accs5[0][:], accs5[1][:], OP.add)
            nc.vector.tensor_tensor(ot[:, 1:2], tmp[:], accs5[2][:], OP.add)
            nc.sync.dma_start(out=out_d[:], in_=ot[:])
    _split_multiwaits(nc)
    return nc


# ---------------------------------------------------------------- host wrapper
_NC_CACHE = None


def _get_nc():
    global _NC_CACHE
    if _NC_CACHE is None:
        _NC_CACHE = build_nc()
    return _NC_CACHE


def prep_core_inputs(xs, hat_xs, core):
    r0 = ROWS_PER_CORE * core
    hat = np.ascontiguousarray(
        hat_xs[r0:r0 + ROWS_PER_CORE]).reshape(128, T, 3)
    phi = np.empty((128, 3, T), np.float32)
    phi[:, :, F_OF_T] = hat.transpose(0, 2, 1)
    xsub = np.ascontiguousarray(
        xs[r0:r0 + ROWS_PER_CORE, ::16, :]).reshape(128, T4, 3)
    xphi = np.empty((128, 3, T4), np.float32)
    xphi[:, :, F4_OF_T4] = xsub.transpose(0, 2, 1)
    return {"phi": phi, "xphi": xphi, "mask": _host_masks()}


def combine(outs):
    s4 = sum(float(o[:, 0].astype(np.float64).sum()) for o in outs)
    s5 = sum(float(o[:, 1].astype(np.float64).sum()) for o in outs)
    loss = W_CONST * HUBER ** 2 * (s4 / CNT4 + 0.5 * s5 / CNT5)
    return np.float32(loss)


def kernel(xs, hat_xs):
    xs = np.asarray(xs, dtype=np.float32)
    hat_xs = np.asarray(hat_xs, dtype=np.float32)
    nc = _get_nc()
    in_maps = [prep_core_inputs(xs, hat_xs, c) for c in range(N_CORES)]
    res = run_bass_kernel_spmd(nc, in_maps, list(range(N_CORES)))
    outs = [res.results[c]["out"] for c in range(N_CORES)]
    return combine(outs)
